# revision 2
# baseline (speedup 1.0000x reference)
"""Trainium2 Bass kernel for AtomWise GNN message passing.

reference:
    rbf_filter = rbf @ w_rbf.T + b_rbf        # [E, C]
    msg = rbf_filter * x                      # [E, C]
    out = segment_sum(msg, edge_index_0, N)   # [N, C]
    out = silu(out @ w1.T + b1); out = silu(out @ w2.T + b2); out = out @ w3.T + b3

Strategy (8 NeuronCores, no collectives):
  - Host: stable-sort edges by destination atom; shard ATOMS (N/8 per core) so
    each core owns all edges of its atom range.  Within a core, atoms are
    processed in 128-atom tiles; each tile's edge list is padded to a global
    E_TILE so every core runs the identical SPMD program.
  - Device (per core, per 512-edge group):
      PE:  filter = rbf_chunk(K=17, bias row folded) @ w_rbfT -> PSUM
      ACT: evacuate filter PSUM -> SBUF bf16
      DVE: msg = filter * x  (bf16 2x mode)
      DVE: one-hot[e, a] = (iota_row == li[e])  (tensor_scalar is_equal, 4x)
      PE:  atom_psum[a, c] += one-hot.T @ msg   (scatter-add as matmul)
    Then per-atom-tile PSUM -> SBUF, PE transposes to [C, atoms] layout and a
    3-layer MLP (f32) runs fully on-chip; output [1, atoms] DMAs out.
"""

import numpy as np

import concourse.bacc as bacc
import concourse.mybir as mybir
import concourse.tile as tile
from concourse.bass_utils import run_bass_kernel_spmd
from concourse.masks import make_identity

N_CORES = 8
P = 128
C = 256
RBF = 16
KF = RBF + 1  # rbf channels + bias row
CHUNK = 128  # edges per scatter matmul (contraction dim)
GROUP_CHUNKS = 4
GROUP_E = CHUNK * GROUP_CHUNKS  # 512 edges per DMA/elementwise group
BF16 = mybir.dt.bfloat16
F32 = mybir.dt.float32
NP_BF16 = mybir.dt.np(BF16)


def _host_prep(x, rbf, num_atoms, edge_index_0, w_rbf, b_rbf):
    """Sort/shard/pad on host. Returns per-core device arrays + dims."""
    E = x.shape[0]
    n_local = num_atoms // N_CORES
    assert num_atoms % N_CORES == 0
    NT = (n_local + P - 1) // P  # atom tiles per core
    A_PAD = NT * P

    idx = np.asarray(edge_index_0).astype(np.int64)
    perm = np.argsort(idx, kind="stable")
    idx_s = idx[perm]
    counts = np.bincount(idx_s, minlength=num_atoms)
    cum = np.concatenate([[0], np.cumsum(counts)])

    # per (core, tile) edge counts -> global E_TILE
    tile_counts = np.zeros((N_CORES, NT), dtype=np.int64)
    for c in range(N_CORES):
        base = c * n_local
        for t in range(NT):
            a0 = base + t * P
            a1 = base + min((t + 1) * P, n_local)
            tile_counts[c, t] = cum[a1] - cum[a0]
    E_TILE = int(-(-tile_counts.max() // GROUP_E) * GROUP_E)
    G_PER_TILE = E_TILE // GROUP_E
    G = NT * G_PER_TILE
    E_PAD = NT * E_TILE
    NCHUNK = E_PAD // CHUNK

    per_core = []
    for c in range(N_CORES):
        base = c * n_local
        lo, hi = cum[base], cum[base + n_local]
        order = perm[lo:hi]
        a_local = (idx_s[lo:hi] - base).astype(np.int64)
        t_id = a_local // P
        # position of each edge within its tile's padded slot range
        tile_start = np.concatenate([[0], np.cumsum(tile_counts[c])])[:-1]
        pos = np.arange(hi - lo) - (cum[base + np.minimum(t_id * P, n_local)] - lo)
        dest = t_id * E_TILE + pos

        xs = np.zeros((E_PAD, C), dtype=np.float32)
        xs[dest] = x[order]
        rbf_pad = np.zeros((E_PAD, KF), dtype=np.float32)
        rbf_pad[dest, :RBF] = rbf[order]
        rbf_pad[dest, RBF] = 1.0
        li = np.full((E_PAD,), -1.0, dtype=np.float32)
        li[dest] = (a_local - t_id * P).astype(np.float32)

        xg = (
            xs.reshape(G, GROUP_CHUNKS, P, C)
            .transpose(0, 2, 1, 3)
            .reshape(G * P, GROUP_CHUNKS * C)
            .astype(NP_BF16)
        )
        rbfT = np.ascontiguousarray(rbf_pad.T).astype(NP_BF16)  # [KF, E_PAD]
        liT = np.ascontiguousarray(li.reshape(NCHUNK, P).T)  # f32
        per_core.append({"xg": xg, "rbfT": rbfT, "liT": liT})

    shared = {
        "wrbfT": np.concatenate(
            [w_rbf.T.astype(np.float32), b_rbf[None].astype(np.float32)], axis=0
        ).astype(NP_BF16),  # [KF, C]
        "iota": np.tile(np.arange(P, dtype=np.float32), (P, 1)).astype(NP_BF16),
    }
    dims = dict(NT=NT, A_PAD=A_PAD, E_TILE=E_TILE, G_PER_TILE=G_PER_TILE, G=G,
                E_PAD=E_PAD, NCHUNK=NCHUNK, n_local=n_local)
    return per_core, shared, dims


def _mlp_weights(w1, b1, w2, b2, w3, b3):
    def wT_blocks(w):  # w [out, in] -> lhsT blocks [P, in//P, out]
        wt = w.T.astype(np.float32)  # [in, out]
        i_dim, o_dim = wt.shape
        return np.ascontiguousarray(
            wt.reshape(i_dim // P, P, o_dim).transpose(1, 0, 2)
        )

    def b_blocks(b):  # [out] -> [P, out//P]
        return np.ascontiguousarray(b.astype(np.float32).reshape(-1, P).T)

    return {
        "w1T": wT_blocks(w1),
        "w2T": wT_blocks(w2),
        "w3T": wT_blocks(w3),  # [P, 2, 1]
        "b1": b_blocks(b1),
        "b2": b_blocks(b2),
    }, float(np.asarray(b3).reshape(-1)[0])


def _build_bass(dims, b3val):
    NT = dims["NT"]
    A_PAD = dims["A_PAD"]
    G_PER_TILE = dims["G_PER_TILE"]
    G = dims["G"]
    E_PAD = dims["E_PAD"]
    NCHUNK = dims["NCHUNK"]
    GC = GROUP_CHUNKS * C  # group free width (1024)

    nc = bacc.Bacc("TRN2", target_bir_lowering=False, debug=False,
                   num_devices=N_CORES)
    xg_d = nc.dram_tensor("xg", [G * P, GC], BF16, kind="ExternalInput")
    rbfT_d = nc.dram_tensor("rbfT", [KF, E_PAD], BF16, kind="ExternalInput")
    liT_d = nc.dram_tensor("liT", [P, NCHUNK], F32, kind="ExternalInput")
    wrbfT_d = nc.dram_tensor("wrbfT", [KF, C], BF16, kind="ExternalInput")
    iota_d = nc.dram_tensor("iota", [P, P], BF16, kind="ExternalInput")
    w1T_d = nc.dram_tensor("w1T", [P, 2, C], F32, kind="ExternalInput")
    w2T_d = nc.dram_tensor("w2T", [P, 2, C], F32, kind="ExternalInput")
    w3T_d = nc.dram_tensor("w3T", [P, 2, 1], F32, kind="ExternalInput")
    b1_d = nc.dram_tensor("b1", [P, 2], F32, kind="ExternalInput")
    b2_d = nc.dram_tensor("b2", [P, 2], F32, kind="ExternalInput")
    y_d = nc.dram_tensor("y", [1, A_PAD], F32, kind="ExternalOutput")

    with tile.TileContext(nc) as tc:
        with (
            tc.tile_pool(name="const", bufs=1) as constp,
            tc.tile_pool(name="pers", bufs=1) as pers,
        ):
            rbfT_sb = constp.tile([KF, E_PAD], BF16)
            nc.sync.dma_start(rbfT_sb[:], rbfT_d[:])
            liT_sb = constp.tile([P, NCHUNK], F32)
            nc.sync.dma_start(liT_sb[:], liT_d[:])
            wrbfT_sb = constp.tile([KF, C], BF16)
            nc.sync.dma_start(wrbfT_sb[:], wrbfT_d[:])
            iota_sb = constp.tile([P, P], BF16)
            nc.sync.dma_start(iota_sb[:], iota_d[:])
            w1T_sb = constp.tile([P, 2, C], F32)
            nc.sync.dma_start(w1T_sb[:], w1T_d[:])
            w2T_sb = constp.tile([P, 2, C], F32)
            nc.sync.dma_start(w2T_sb[:], w2T_d[:])
            w3T_sb = constp.tile([P, 2, 1], F32)
            nc.sync.dma_start(w3T_sb[:], w3T_d[:])
            b1_sb = constp.tile([P, 2], F32)
            nc.sync.dma_start(b1_sb[:], b1_d[:])
            b2_sb = constp.tile([P, 2], F32)
            nc.sync.dma_start(b2_sb[:], b2_d[:])
            ident_sb = constp.tile([P, P], F32)
            make_identity(nc, ident_sb[:])

            h0_all = pers.tile([P, NT * C], F32)  # scatter result [atoms, C]
            hT = pers.tile([P, 2, A_PAD], F32)  # transposed [C, atoms]
            h1T = pers.tile([P, 2, A_PAD], F32)
            h2T = pers.tile([P, 2, A_PAD], F32)
            y_sb = pers.tile([1, A_PAD], F32)

            # ---- phase 1: filter, msg, scatter ----
            with (
                tc.tile_pool(name="xt", bufs=3) as xtp,
                tc.tile_pool(name="fsb", bufs=3) as fsbp,
                tc.tile_pool(name="msg", bufs=3) as msgp,
                tc.tile_pool(name="oh", bufs=8) as ohp,
                tc.tile_pool(name="fps", bufs=2, space="PSUM") as fpsp,
                tc.tile_pool(name="sps", bufs=2, space="PSUM") as spsp,
            ):
                for t in range(NT):
                    spsum = spsp.tile([P, C], F32)
                    for gt in range(G_PER_TILE):
                        g = t * G_PER_TILE + gt
                        xt = xtp.tile([P, GC], BF16)
                        nc.sync.dma_start(xt[:], xg_d[g * P:(g + 1) * P, :])
                        fps = fpsp.tile([P, GC], F32)
                        for q in range(GROUP_CHUNKS):
                            ch = g * GROUP_CHUNKS + q
                            nc.tensor.matmul(
                                fps[:, q * C:(q + 1) * C],
                                lhsT=rbfT_sb[:, ch * P:(ch + 1) * P],
                                rhs=wrbfT_sb[:],
                                start=True,
                                stop=True,
                            )
                        fsb = fsbp.tile([P, GC], BF16)
                        nc.scalar.activation(
                            fsb[:], fps[:], mybir.ActivationFunctionType.Copy
                        )
                        msg = msgp.tile([P, GC], BF16)
                        nc.vector.tensor_tensor(
                            out=msg[:], in0=fsb[:], in1=xt[:],
                            op=mybir.AluOpType.mult,
                        )
                        for q in range(GROUP_CHUNKS):
                            ch = g * GROUP_CHUNKS + q
                            oh = ohp.tile([P, P], BF16)
                            nc.vector.tensor_scalar(
                                oh[:], iota_sb[:], liT_sb[:, ch:ch + 1], None,
                                mybir.AluOpType.is_equal,
                            )
                            nc.tensor.matmul(
                                spsum[:],
                                lhsT=oh[:],
                                rhs=msg[:, q * C:(q + 1) * C],
                                start=(gt == 0 and q == 0),
                                stop=(gt == G_PER_TILE - 1 and q == GROUP_CHUNKS - 1),
                            )
                    nc.vector.tensor_copy(h0_all[:, t * C:(t + 1) * C], spsum[:])

            # ---- phase 2: transpose + MLP ----
            with (
                tc.tile_pool(name="tps", bufs=2, space="PSUM") as tpsp,
                tc.tile_pool(name="mps", bufs=2, space="PSUM") as mpsp,
            ):
                for t in range(NT):
                    for k in range(2):
                        tps = tpsp.tile([P, P], F32)
                        nc.tensor.transpose(
                            tps[:],
                            h0_all[:, t * C + k * P: t * C + (k + 1) * P],
                            ident_sb[:],
                        )
                        nc.vector.tensor_copy(hT[:, k, t * P:(t + 1) * P], tps[:])

                def mlp_layer(src, dst, wsb, bsb):
                    for m in range(2):
                        for n0 in range(0, A_PAD, 512):
                            nsz = min(512, A_PAD - n0)
                            mp = mpsp.tile([P, 512], F32)
                            for k in range(2):
                                nc.tensor.matmul(
                                    mp[:, :nsz],
                                    lhsT=wsb[:, k, m * P:(m + 1) * P],
                                    rhs=src[:, k, n0:n0 + nsz],
                                    start=(k == 0),
                                    stop=(k == 1),
                                )
                            nc.scalar.activation(
                                dst[:, m, n0:n0 + nsz],
                                mp[:, :nsz],
                                mybir.ActivationFunctionType.Silu,
                                bias=bsb[:, m:m + 1],
                            )

                mlp_layer(hT, h1T, w1T_sb, b1_sb)
                mlp_layer(h1T, h2T, w2T_sb, b2_sb)

                for n0 in range(0, A_PAD, 512):
                    nsz = min(512, A_PAD - n0)
                    mp = mpsp.tile([P, 512], F32)
                    for k in range(2):
                        nc.tensor.matmul(
                            mp[:1, :nsz],
                            lhsT=w3T_sb[:, k, :],
                            rhs=h2T[:, k, n0:n0 + nsz],
                            start=(k == 0),
                            stop=(k == 1),
                        )
                    nc.scalar.activation(
                        y_sb[:, n0:n0 + nsz],
                        mp[:1, :nsz],
                        mybir.ActivationFunctionType.Copy,
                        bias=b3val,
                    )
                nc.sync.dma_start(y_d[:], y_sb[:])

    nc.compile()
    return nc


def _prepare(x, rbf, num_atoms, edge_index_0, w_rbf, b_rbf, w1, b1, w2, b2, w3, b3):
    x = np.asarray(x, dtype=np.float32)
    rbf = np.asarray(rbf, dtype=np.float32)
    num_atoms = int(num_atoms)
    per_core, shared, dims = _host_prep(x, rbf, num_atoms, edge_index_0,
                                        np.asarray(w_rbf, np.float32),
                                        np.asarray(b_rbf, np.float32))
    mlp, b3val = _mlp_weights(
        np.asarray(w1, np.float32), np.asarray(b1, np.float32),
        np.asarray(w2, np.float32), np.asarray(b2, np.float32),
        np.asarray(w3, np.float32), np.asarray(b3, np.float32))
    nc = _build_bass(dims, b3val)
    in_maps = [{**pc, **shared, **mlp} for pc in per_core]
    return nc, in_maps, dims


def kernel(**inputs) -> np.ndarray:
    num_atoms = int(inputs["num_atoms"])
    nc, in_maps, dims = _prepare(**inputs)
    res = run_bass_kernel_spmd(nc, in_maps, core_ids=list(range(N_CORES)))
    n_local = dims["n_local"]
    out = np.empty((num_atoms, 1), dtype=np.float32)
    for c in range(N_CORES):
        out[c * n_local:(c + 1) * n_local, 0] = res.results[c]["y"][0, :n_local]
    return out


# revision 14
# speedup vs baseline: 1.5054x; 1.5054x over previous
"""Trainium2 Bass kernel for AtomWise GNN message passing.

reference:
    rbf_filter = rbf @ w_rbf.T + b_rbf        # [E, C]
    msg = rbf_filter * x                      # [E, C]
    out = segment_sum(msg, edge_index_0, N)   # [N, C]
    out = silu(out @ w1.T + b1); out = silu(out @ w2.T + b2); out = out @ w3.T + b3

Strategy (8 NeuronCores, no collectives):
  - Host: stable-sort edges by destination atom; shard ATOMS (N/8 per core) so
    each core owns all edges of its atom range.  Within a core, atoms are
    processed in 128-atom tiles; each tile's edge list is padded to a global
    E_TILE so every core runs the identical SPMD program.
  - Device (per core, per 512-edge group):
      PE:  filter = rbf_chunk(K=17, bias row folded) @ w_rbfT -> PSUM
           (4 chunks packed into row-groups via tile_position, running
            concurrently on 32-row strips of the PE array)
      ACT: evacuate filter PSUM -> SBUF bf16
      DVE: msg = filter * x  (bf16 2x mode)
      DVE: one-hot[e, a] = (iota_row == li[e])  (tensor_scalar is_equal, 4x)
      PE:  atom_psum[a, c] += one-hot.T @ msg   (scatter-add as matmul)
    Then per-atom-tile PSUM -> SBUF, PE transposes to [C, atoms] layout and a
    3-layer MLP (bf16 matmuls, f32 accumulate) runs on-chip; output [1, atoms].
"""

import numpy as np

import concourse.bacc as bacc
import concourse.mybir as mybir
import concourse.tile as tile
from concourse.bass_utils import run_bass_kernel_spmd
from concourse.masks import make_identity

N_CORES = 8
P = 128
C = 256
RBF = 16
KF = RBF + 1  # rbf channels + bias row
CHUNK = 128  # edges per scatter matmul (contraction dim)
GROUP_CHUNKS = 4
GROUP_E = CHUNK * GROUP_CHUNKS  # 512 edges per elementwise group
DMA_GROUPS = 4  # groups per x DMA (2048 edges, 1 MiB)
DMA_E = GROUP_E * DMA_GROUPS
BF16 = mybir.dt.bfloat16
F32 = mybir.dt.float32
NP_BF16 = mybir.dt.np(BF16)

PACK_FILTER = True  # tile_position row-packing of the 4 K=17 filter matmuls


def _host_prep(x, rbf, num_atoms, edge_index_0, w_rbf, b_rbf):
    """Sort/shard/pad on host. Returns per-core device arrays + dims."""
    n_local = num_atoms // N_CORES
    assert num_atoms % N_CORES == 0
    NT = (n_local + P - 1) // P  # atom tiles per core
    A_PAD = NT * P

    idx = np.asarray(edge_index_0).astype(np.int64)
    perm = np.argsort(idx, kind="stable")
    idx_s = idx[perm]
    counts = np.bincount(idx_s, minlength=num_atoms)
    cum = np.concatenate([[0], np.cumsum(counts)])

    # per (core, tile) edge counts -> global E_TILE (multiple of DMA_E)
    tile_counts = np.zeros((N_CORES, NT), dtype=np.int64)
    for c in range(N_CORES):
        base = c * n_local
        for t in range(NT):
            a0 = base + t * P
            a1 = base + min((t + 1) * P, n_local)
            tile_counts[c, t] = cum[a1] - cum[a0]
    E_TILE = int(-(-tile_counts.max() // DMA_E) * DMA_E)
    G_PER_TILE = E_TILE // GROUP_E
    G = NT * G_PER_TILE
    E_PAD = NT * E_TILE
    NCHUNK = E_PAD // CHUNK
    D = E_PAD // DMA_E  # number of x DMAs

    per_core = []
    for c in range(N_CORES):
        base = c * n_local
        lo, hi = cum[base], cum[base + n_local]
        order = perm[lo:hi]
        a_local = (idx_s[lo:hi] - base).astype(np.int64)
        t_id = a_local // P
        # position of each edge within its tile's padded slot range
        pos = np.arange(hi - lo) - (cum[base + t_id * P] - lo)
        dest = t_id * E_TILE + pos

        xs = np.zeros((E_PAD, C), dtype=np.float32)
        xs[dest] = x[order]
        rbf_pad = np.zeros((E_PAD, KF), dtype=np.float32)
        rbf_pad[dest, :RBF] = rbf[order]
        rbf_pad[dest, RBF] = 1.0
        li = np.full((E_PAD,), -1.0, dtype=np.float32)
        li[dest] = (a_local - t_id * P).astype(np.float32)

        # x: [D, (4 dma-groups, 4 chunks), 128, C] -> [D*128, 16*C]
        # with PACK_FILTER, chunks within a group are stored in the psum
        # evacuation order [0, 2, 1, 3]
        xs4 = xs.reshape(D, DMA_GROUPS, GROUP_CHUNKS, P, C)
        if PACK_FILTER:
            xs4 = xs4[:, :, [0, 2, 1, 3]]
        xg = (
            xs4.reshape(D, DMA_GROUPS * GROUP_CHUNKS, P, C)
            .transpose(0, 2, 1, 3)
            .reshape(D * P, DMA_GROUPS * GROUP_CHUNKS * C)
            .astype(NP_BF16)
        )
        if PACK_FILTER:
            # rbfT packed for 2-row-group tiling: chunk (g,q) on partitions
            # [32*(q%2), +KF), columns [g*256 + (q//2)*128, +128)
            arr = rbf_pad.reshape(G, GROUP_CHUNKS, P, KF)
            rbfT = np.zeros((P, G, 2, P), dtype=np.float32)
            for q in range(GROUP_CHUNKS):
                rbfT[32 * (q % 2):32 * (q % 2) + KF, :, q // 2, :] = (
                    arr[:, q].transpose(2, 0, 1)
                )
            rbfT = rbfT.reshape(P, G * 2 * P).astype(NP_BF16)
        else:
            rbfT = np.ascontiguousarray(rbf_pad.T).astype(NP_BF16)  # [KF, E_PAD]
        liT = np.ascontiguousarray(li.reshape(NCHUNK, P).T)  # f32 [P, NCHUNK]
        per_core.append({"xg": xg, "rbfT": rbfT, "liT": liT})

    wrbfT = np.concatenate(
        [w_rbf.T.astype(np.float32), b_rbf[None].astype(np.float32)], axis=0
    )  # [KF, C]
    if PACK_FILTER:
        w4 = np.zeros((P, C), dtype=np.float32)
        for rg in (0, 32):
            w4[rg:rg + KF] = wrbfT
        wrbfT = w4
    shared = {
        "wrbfT": wrbfT.astype(NP_BF16),
        "iota": np.tile(np.arange(P, dtype=np.float32), (P, 1)).astype(NP_BF16),
    }
    dims = dict(NT=NT, A_PAD=A_PAD, E_TILE=E_TILE, G_PER_TILE=G_PER_TILE, G=G,
                E_PAD=E_PAD, NCHUNK=NCHUNK, n_local=n_local, D=D)
    return per_core, shared, dims


def _mlp_weights(w1, b1, w2, b2, w3, b3):
    def wT_blocks(w):  # w [out, in] -> lhsT blocks [P, in//P, out]
        wt = w.T.astype(np.float32)  # [in, out]
        i_dim, o_dim = wt.shape
        return np.ascontiguousarray(
            wt.reshape(i_dim // P, P, o_dim).transpose(1, 0, 2)
        ).astype(NP_BF16)

    def b_blocks(b):  # [out] -> [P, out//P]
        return np.ascontiguousarray(b.astype(np.float32).reshape(-1, P).T)

    return {
        "w1T": wT_blocks(w1),
        "w2T": wT_blocks(w2),
        "w3T": wT_blocks(w3),  # [P, 2, 1]
        "b1": b_blocks(b1),
        "b2": b_blocks(b2),
    }, float(np.asarray(b3).reshape(-1)[0])


def _build_bass(dims, b3val):
    NT = dims["NT"]
    A_PAD = dims["A_PAD"]
    G_PER_TILE = dims["G_PER_TILE"]
    G = dims["G"]
    E_PAD = dims["E_PAD"]
    NCHUNK = dims["NCHUNK"]
    D = dims["D"]
    GC = GROUP_CHUNKS * C  # elementwise group width (1024)
    XC = DMA_GROUPS * GC  # x DMA tile width (4096)
    DPT = G_PER_TILE // DMA_GROUPS  # x DMAs per atom tile

    nc = bacc.Bacc("TRN2", target_bir_lowering=False, debug=False,
                   num_devices=N_CORES)
    xg_d = nc.dram_tensor("xg", [D * P, XC], BF16, kind="ExternalInput")
    rbf_shape = [P, G * 2 * P] if PACK_FILTER else [KF, E_PAD]
    rbfT_d = nc.dram_tensor("rbfT", rbf_shape, BF16, kind="ExternalInput")
    liT_d = nc.dram_tensor("liT", [P, NCHUNK], F32, kind="ExternalInput")
    wrbf_shape = [P, C] if PACK_FILTER else [KF, C]
    wrbfT_d = nc.dram_tensor("wrbfT", wrbf_shape, BF16, kind="ExternalInput")
    iota_d = nc.dram_tensor("iota", [P, P], BF16, kind="ExternalInput")
    w1T_d = nc.dram_tensor("w1T", [P, 2, C], BF16, kind="ExternalInput")
    w2T_d = nc.dram_tensor("w2T", [P, 2, C], BF16, kind="ExternalInput")
    w3T_d = nc.dram_tensor("w3T", [P, 2, 1], BF16, kind="ExternalInput")
    b1_d = nc.dram_tensor("b1", [P, 2], F32, kind="ExternalInput")
    b2_d = nc.dram_tensor("b2", [P, 2], F32, kind="ExternalInput")
    y_d = nc.dram_tensor("y", [1, A_PAD], F32, kind="ExternalOutput")

    with tile.TileContext(nc) as tc:
        with (
            tc.tile_pool(name="const", bufs=1) as constp,
            tc.tile_pool(name="pers", bufs=1) as pers,
            tc.tile_pool(name="xt", bufs=3) as xtp,
            tc.tile_pool(name="fsb", bufs=4) as fsbp,
            tc.tile_pool(name="msg", bufs=4) as msgp,
            tc.tile_pool(name="oh", bufs=16) as ohp,
            tc.tile_pool(name="fps", bufs=2, space="PSUM") as fpsp,
            tc.tile_pool(name="sps", bufs=2, space="PSUM") as spsp,
            tc.tile_pool(name="tps", bufs=1, space="PSUM") as tpsp,
            tc.tile_pool(name="mps", bufs=1, space="PSUM") as mpsp,
        ):
            # --- constants: order matters for startup latency ---
            iota_sb = constp.tile([P, P], BF16)
            nc.sync.dma_start(iota_sb[:], iota_d[:])
            wrbfT_sb = constp.tile(wrbf_shape, BF16)
            nc.sync.dma_start(wrbfT_sb[:], wrbfT_d[:])
            rbfT_sb = constp.tile(rbf_shape, BF16)
            head = min(rbf_shape[1], 2 * (rbf_shape[1] // NT))
            nc.sync.dma_start(rbfT_sb[:, :head], rbfT_d[:, :head])

            xts = {}
            fpss = {}
            spsums = {}

            def emit_dma(d):
                xt = xtp.tile([P, XC], BF16, name="xt", tag="xt")
                nc.sync.dma_start(xt[:], xg_d[d * P:(d + 1) * P, :])
                xts[d] = xt

            def emit_filter(g):
                if PACK_FILTER:
                    fps = fpsp.tile([P, 2, 512], F32, name="fps", tag="fps")
                else:
                    fps = fpsp.tile([P, GC], F32, name="fps", tag="fps")
                for q in range(GROUP_CHUNKS):
                    ch = g * GROUP_CHUNKS + q
                    if PACK_FILTER:
                        rg = 32 * (q % 2)
                        nc.tensor.matmul(
                            fps[:, q % 2, (q // 2) * C:(q // 2 + 1) * C],
                            lhsT=rbfT_sb[rg:rg + KF,
                                         g * 2 * P + (q // 2) * P:
                                         g * 2 * P + (q // 2 + 1) * P],
                            rhs=wrbfT_sb[rg:rg + KF, :],
                            start=True,
                            stop=True,
                            tile_position=(rg, 0),
                        )
                    else:
                        nc.tensor.matmul(
                            fps[:, q * C:(q + 1) * C],
                            lhsT=rbfT_sb[:, ch * P:(ch + 1) * P],
                            rhs=wrbfT_sb[:],
                            start=True,
                            stop=True,
                        )
                fpss[g] = fps

            def emit_consume(g):
                t, gt = divmod(g, G_PER_TILE)
                if gt == 0:
                    spsums[t] = spsp.tile([P, C], F32, name="spsum", tag="sps")
                spsum = spsums[t]
                fps = fpss.pop(g)
                xt = xts[g // DMA_GROUPS]
                g2 = g % DMA_GROUPS
                fsb = fsbp.tile([P, GC], BF16, name="fsb", tag="fsb")
                fps_ap = fps[:] if PACK_FILTER else fps[:]
                if g % 5 == 2:
                    nc.vector.tensor_copy(fsb[:], fps_ap)
                else:
                    nc.scalar.activation(
                        fsb[:], fps_ap, mybir.ActivationFunctionType.Copy,
                    )
                msg = msgp.tile([P, GC], BF16, name="msg", tag="msg")
                nc.vector.tensor_tensor(
                    out=msg[:], in0=fsb[:],
                    in1=xt[:, g2 * GC:(g2 + 1) * GC],
                    op=mybir.AluOpType.mult,
                )
                for q in range(GROUP_CHUNKS):
                    ch = g * GROUP_CHUNKS + q
                    oh = ohp.tile([P, P], BF16, name="oh", tag="oh")
                    oh_eng = nc.vector if ch % 8 == 7 else nc.gpsimd
                    oh_eng.tensor_scalar(
                        oh[:], iota_sb[:], liT_sb[:, ch:ch + 1], None,
                        mybir.AluOpType.is_equal,
                    )
                    pq = (2 * (q % 2) + q // 2) if PACK_FILTER else q
                    nc.tensor.matmul(
                        spsum[:],
                        lhsT=oh[:],
                        rhs=msg[:, pq * C:(pq + 1) * C],
                        start=(gt == 0 and q == 0),
                        stop=(gt == G_PER_TILE - 1 and q == GROUP_CHUNKS - 1),
                    )

            def emit_tile_end(t):
                nc.any.tensor_copy(h0_all[:, t * C:(t + 1) * C],
                                   spsums.pop(t)[:])
                for k in range(2):
                    tps = tpsp.tile([P, P], BF16, name="tps", tag="tps")
                    nc.tensor.transpose(
                        tps[:],
                        h0_all[:, t * C + k * P: t * C + (k + 1) * P],
                        ident_sb[:],
                    )
                    nc.any.tensor_copy(hT[:, k, t * P:(t + 1) * P], tps[:])

            def emit_mlp_chunk(n0):
                nsz = min(512, A_PAD - n0)

                def layer(src_t, dst, wsb, bsb):
                    mp = mpsp.tile([P, 512], F32, name="mp", tag="mp")
                    for k in range(2):
                        nc.tensor.matmul(
                            mp[:, :nsz],
                            lhsT=wsb[:, k, :] if wsb is w3T_sb
                            else wsb[:, k, 0:P],
                            rhs=src_t[:, k, n0:n0 + nsz],
                            start=(k == 0),
                            stop=(k == 1),
                        )
                    return mp

                for m in range(2):
                    mp = mpsp.tile([P, 512], F32, name="mp", tag="mp")
                    for k in range(2):
                        nc.tensor.matmul(
                            mp[:, :nsz],
                            lhsT=w1T_sb[:, k, m * P:(m + 1) * P],
                            rhs=hT[:, k, n0:n0 + nsz],
                            start=(k == 0), stop=(k == 1),
                        )
                    nc.scalar.activation(
                        h1T[:, m, n0:n0 + nsz], mp[:, :nsz],
                        mybir.ActivationFunctionType.Silu,
                        bias=b1_sb[:, m:m + 1],
                    )
                for m in range(2):
                    mp = mpsp.tile([P, 512], F32, name="mp", tag="mp")
                    for k in range(2):
                        nc.tensor.matmul(
                            mp[:, :nsz],
                            lhsT=w2T_sb[:, k, m * P:(m + 1) * P],
                            rhs=h1T[:, k, n0:n0 + nsz],
                            start=(k == 0), stop=(k == 1),
                        )
                    nc.scalar.activation(
                        h2T[:, m, n0:n0 + nsz], mp[:, :nsz],
                        mybir.ActivationFunctionType.Silu,
                        bias=b2_sb[:, m:m + 1],
                    )
                mp = mpsp.tile([P, 512], F32, name="mp", tag="mp")
                for k in range(2):
                    nc.tensor.matmul(
                        mp[:1, :nsz],
                        lhsT=w3T_sb[:, k, :],
                        rhs=h2T[:, k, n0:n0 + nsz],
                        start=(k == 0), stop=(k == 1),
                    )
                nc.scalar.activation(
                    y_sb[:, n0:n0 + nsz], mp[:1, :nsz],
                    mybir.ActivationFunctionType.Copy, bias=b3val,
                )

            # --- pipelined emission ---
            emit_dma(0)
            emit_filter(0)

            # remaining constants (needed later; after the first x tile)
            liT_sb = constp.tile([P, NCHUNK], F32)
            nc.sync.dma_start(liT_sb[:], liT_d[:])
            if head < rbf_shape[1]:
                nc.sync.dma_start(rbfT_sb[:, head:], rbfT_d[:, head:])
            w1T_sb = constp.tile([P, 2, C], BF16)
            nc.sync.dma_start(w1T_sb[:], w1T_d[:])
            w2T_sb = constp.tile([P, 2, C], BF16)
            nc.sync.dma_start(w2T_sb[:], w2T_d[:])
            w3T_sb = constp.tile([P, 2, 1], BF16)
            nc.sync.dma_start(w3T_sb[:], w3T_d[:])
            b1_sb = constp.tile([P, 2], F32)
            nc.sync.dma_start(b1_sb[:], b1_d[:])
            b2_sb = constp.tile([P, 2], F32)
            nc.sync.dma_start(b2_sb[:], b2_d[:])
            ident_sb = constp.tile([P, P], BF16)
            make_identity(nc, ident_sb[:])

            h0_all = pers.tile([P, NT * C], BF16)
            hT = pers.tile([P, 2, A_PAD], BF16)
            h1T = pers.tile([P, 2, A_PAD], BF16)
            h2T = pers.tile([P, 2, A_PAD], BF16)
            y_sb = pers.tile([1, A_PAD], F32)

            mlp_points = {}  # last tile index -> list of n0 chunks ready
            for n0 in range(0, A_PAD, 512):
                nsz = min(512, A_PAD - n0)
                t_req = (n0 + nsz - 1) // P
                mlp_points.setdefault(t_req, []).append(n0)

            for g in range(G):
                if (g + 1) % DMA_GROUPS == 0 and g + 1 < G:
                    emit_dma((g + 1) // DMA_GROUPS)
                if g + 1 < G:
                    emit_filter(g + 1)
                emit_consume(g)
                t, gt = divmod(g, G_PER_TILE)
                if gt == G_PER_TILE - 1:
                    emit_tile_end(t)
                    for n0 in mlp_points.get(t, []):
                        emit_mlp_chunk(n0)
            nc.sync.dma_start(y_d[:], y_sb[:])

    nc.compile()
    return nc


def _prepare(x, rbf, num_atoms, edge_index_0, w_rbf, b_rbf, w1, b1, w2, b2, w3, b3):
    x = np.asarray(x, dtype=np.float32)
    rbf = np.asarray(rbf, dtype=np.float32)
    num_atoms = int(num_atoms)
    per_core, shared, dims = _host_prep(x, rbf, num_atoms, edge_index_0,
                                        np.asarray(w_rbf, np.float32),
                                        np.asarray(b_rbf, np.float32))
    mlp, b3val = _mlp_weights(
        np.asarray(w1, np.float32), np.asarray(b1, np.float32),
        np.asarray(w2, np.float32), np.asarray(b2, np.float32),
        np.asarray(w3, np.float32), np.asarray(b3, np.float32))
    nc = _build_bass(dims, b3val)
    in_maps = [{**pc, **shared, **mlp} for pc in per_core]
    return nc, in_maps, dims


def kernel(**inputs) -> np.ndarray:
    num_atoms = int(inputs["num_atoms"])
    nc, in_maps, dims = _prepare(**inputs)
    res = run_bass_kernel_spmd(nc, in_maps, core_ids=list(range(N_CORES)))
    n_local = dims["n_local"]
    out = np.empty((num_atoms, 1), dtype=np.float32)
    for c in range(N_CORES):
        out[c * n_local:(c + 1) * n_local, 0] = res.results[c]["y"][0, :n_local]
    return out
